# revision 19
# baseline (speedup 1.0000x reference)
"""Trainium2 Bass kernel for nn_CustomConv_66769561583718.

Reference op (per batch element):
  out = conv(x, W, stride=2, dilation=2, VALID)            # 3x3 taps, 9 total
      + conv(x, bias1[o] * FUZ, stride=2, VALID)           # dense 5x5

Structure exploited:
  * Term 1 reads only even-even input pixels; it is expressed as 9
    shifted 1x1 convs (matmuls over the 128 input channels) accumulated
    in PSUM.
  * FUZ = 0.1 * (ones(5,5) - dilated3x3_ones), and the 5x5 kernel is
    constant over input channels, so term 2 is rank-1:
        term2[o, y, x] = bias1[o] * S[y, x]
        S = 0.1 * (box5(T) - dilated3(T)) at stride 2,
        T[h, w] = sum_c x[c, h, w]
    T comes from M=1 ones-matmuls; the separable box passes are DVE adds
    (w direction) and a small matmul (h direction); the rank-1 term is a
    K=1 matmul accumulated into the same PSUM group as term 1.

Sharding: data-parallel over batch, 2 batches per core on 8 cores.

Emission order is tuned so PE never waits long on the big input DMA:
x arrives in h-chunks and T-matmuls + conv tap groups are interleaved
to track chunk arrival. Bias matmuls (which need the fully-reduced S)
close each PSUM group late, once s_row is ready.
"""

import numpy as np

import concourse.bacc as bacc
import concourse.mybir as mybir
import concourse.tile as tile
from concourse.bass_utils import run_bass_kernel_spmd

import ml_dtypes

dt = mybir.dt

B, CIN, H, W = 16, 128, 112, 112
COUT = 256
N_CORES = 8
BPC = B // N_CORES          # batches per core
HO = WO = 54
HW = H * W                  # 12544
T_CHUNK = 448               # 4 h-rows; 28 chunks per batch
N_TCHUNK = HW // T_CHUNK    # 28
# h-row boundaries of the input DMA chunks (small first chunk so PE
# starts early; the rest big for DMA efficiency)
X_CHUNK_ROWS = [0, 4, 16, 32, 48, 64, 80, 96, 112]
N_XCHUNK = len(X_CHUNK_ROWS) - 1
YT = 9                      # y-rows per output tile
NTILE = HO // YT            # 6 tiles per (batch, half)
NSP = YT * WO               # 486 spatial positions per tile

# conv/T datapath dtype: dt.float32r (fp32-precision-ish, 1 col/cycle at
# N>=256) or dt.bfloat16 (halves input HBM traffic; host pre-casts)
CONV_DT = dt.bfloat16
# output dtype: dt.float32 (exact) or dt.bfloat16 (halves output HBM
# traffic; host upcasts after gather)
OUT_DT = dt.float32


def _np_conv_dt(conv_dt):
    return ml_dtypes.bfloat16 if conv_dt == dt.bfloat16 else np.float32


# ablation switch for benching: subset of {"t","s","c"}; "c" = conv taps,
# "t" = channel-sum matmuls, "s" = S chain + bias matmuls (needs "t")
_PARTS = "tsc"


def _build(conv_dt=CONV_DT, iters=1, parts=None):
    if parts is None:
        parts = _PARTS
    do_t = "t" in parts
    do_s = "s" in parts and do_t
    do_c = "c" in parts
    nc = bacc.Bacc(None, target_bir_lowering=False)

    x = nc.dram_tensor("x", [BPC, CIN, H, W], conv_dt, kind="ExternalInput")
    # wt[c, tap, o] = weight[o, c, ky, kx], tap = ky*3+kx
    wt = nc.dram_tensor("wt", [CIN, 9, COUT], conv_dt, kind="ExternalInput")
    bias = nc.dram_tensor("bias", [1, COUT], dt.bfloat16, kind="ExternalInput")
    # lmat[h, k*HO + y]: k=0 -> 0.1*[2y<=h<=2y+4], k=1 -> -0.1*[h-2y in 0,2,4]
    lmat = nc.dram_tensor("lmat", [H, 2 * HO], dt.float32, kind="ExternalInput")
    ones = nc.dram_tensor("ones", [CIN, 1], conv_dt, kind="ExternalInput")
    out = nc.dram_tensor("out", [BPC, COUT, HO, WO], OUT_DT, kind="ExternalOutput")

    with tile.TileContext(nc) as tc:
        with (
            tc.tile_pool(name="const", bufs=1) as cpool,
            tc.tile_pool(name="x", bufs=1) as xpool,
            tc.tile_pool(name="trow", bufs=1) as trowpool,
            tc.tile_pool(name="small", bufs=2) as spool,
            tc.tile_pool(name="outsb", bufs=4) as opool,
            tc.tile_pool(name="pts", bufs=2, space="PSUM") as pts,
            tc.tile_pool(name="psc", bufs=6, space="PSUM") as psc,
        ):
            x_sbs = [
                xpool.tile([CIN, HW], conv_dt, tag=f"x{b}", name=f"x_sb{b}")
                for b in range(BPC)
            ]

            def emit_x_chunks(b, cs):
                xc = x[b].rearrange("c h w -> c (h w)")
                for c in cs:
                    lo = X_CHUNK_ROWS[c] * W
                    hi = X_CHUNK_ROWS[c + 1] * W
                    nc.sync.dma_start(out=x_sbs[b][:, lo:hi], in_=xc[:, lo:hi])

            # batch0 chunk 0 first so PE starts ASAP, then consts + weights,
            # then the rest of batch0. Later batches prefetch inside the loop.
            ones_sb = cpool.tile([CIN, 1], conv_dt)
            nc.sync.dma_start(out=ones_sb[:], in_=ones[:])

            def emit_consts():
                bias_sb = cpool.tile([1, COUT], dt.bfloat16)
                nc.sync.dma_start(out=bias_sb[:], in_=bias[:])
                lmat_sb = cpool.tile([H, 2 * HO], dt.float32)
                nc.sync.dma_start(out=lmat_sb[:], in_=lmat[:])
                wt_sb = cpool.tile([CIN, 9 * COUT], conv_dt)
                nc.sync.dma_start(
                    out=wt_sb[:], in_=wt[:].rearrange("c t o -> c (t o)")
                )
                return bias_sb, lmat_sb, wt_sb

            def emit_body(bias_sb, lmat_sb, wt_sb):
                for b in range(BPC):
                    x_sb = x_sbs[b]
                    xv = x_sb[:].rearrange("c (h w) -> c h w", w=W)

                    t_row = trowpool.tile([1, HW], dt.float32, tag="t_row",
                                          name="t_row")

                    def emit_t_chunks(ks):
                        for k in ks:
                            pt = pts.tile([1, T_CHUNK], dt.float32, tag="ts",
                                          name="pt")
                            nc.tensor.matmul(
                                out=pt[:],
                                lhsT=ones_sb[:],
                                rhs=x_sb[:, k * T_CHUNK : (k + 1) * T_CHUNK],
                                start=True,
                                stop=True,
                            )
                            sl = t_row[0:1, k * T_CHUNK : (k + 1) * T_CHUNK]
                            if k % 2 == 0:
                                nc.scalar.copy(out=sl, in_=pt[:])
                            else:
                                nc.vector.tensor_copy(out=sl, in_=pt[:])

                    def emit_taps(half, ti):
                        y0 = ti * YT
                        pc = psc.tile([128, NSP], dt.float32, tag="pc", name="pc")
                        for tap in range(9):
                            ky, kx = divmod(tap, 3)
                            h0 = 2 * y0 + 2 * ky
                            nc.tensor.matmul(
                                out=pc[:],
                                lhsT=wt_sb[
                                    :,
                                    tap * COUT + half * 128 :
                                    tap * COUT + half * 128 + 128,
                                ],
                                rhs=xv[
                                    :, h0 : h0 + 17 : 2, 2 * kx : 2 * kx + 107 : 2
                                ],
                                start=(tap == 0),
                                stop=(tap == 8 and not do_s),
                            )
                        return pc

                    def emit_bias_and_evict(half, ti, pc, s_row):
                        y0 = ti * YT
                        if do_s:
                            nc.tensor.matmul(
                                out=pc[:],
                                lhsT=bias_sb[0:1, half * 128 : half * 128 + 128],
                                rhs=s_row[0:1, y0 * WO : (y0 + YT) * WO],
                                start=False,
                                stop=True,
                            )
                        o_sb = opool.tile([128, NSP], OUT_DT, name="o_sb")
                        nc.vector.tensor_copy(out=o_sb[:], in_=pc[:])
                        nc.sync.dma_start(
                            out=out[
                                b, half * 128 : half * 128 + 128, y0 : y0 + YT, :
                            ],
                            in_=o_sb[:],
                        )

                    t_hw = spool.tile([H, W], dt.float32, tag="t_hw", name="t_hw")
                    c53 = spool.tile([H, 2 * HO], dt.float32, tag="c53", name="c53")
                    tmp = spool.tile([H, WO], dt.float32, tag="ctmp", name="tmp")

                    def emit_t_half(r0, r1):
                        # reshape rows [r0:r1] of T and run the w-direction
                        # box passes on them (C5 | C3 into c53)
                        nc.sync.dma_start(
                            out=t_hw[r0:r1, :], in_=t_row[0:1, r0 * W : r1 * W]
                        )
                        t = t_hw
                        nc.vector.tensor_add(
                            out=tmp[r0:r1, :],
                            in0=t[r0:r1, 0:107:2],
                            in1=t[r0:r1, 2:109:2],
                        )
                        nc.vector.tensor_add(
                            out=c53[r0:r1, HO : 2 * HO],
                            in0=tmp[r0:r1, :],
                            in1=t[r0:r1, 4:111:2],
                        )
                        nc.vector.tensor_add(
                            out=tmp[r0:r1, :],
                            in0=t[r0:r1, 1:108:2],
                            in1=t[r0:r1, 3:110:2],
                        )
                        nc.vector.tensor_add(
                            out=c53[r0:r1, 0:HO],
                            in0=c53[r0:r1, HO : 2 * HO],
                            in1=tmp[r0:r1, :],
                        )

                    def emit_s_chain():
                        ps_s = pts.tile([HO, WO], dt.float32, tag="ts", name="ps_s")
                        nc.tensor.matmul(
                            out=ps_s[:],
                            lhsT=lmat_sb[:, 0:HO],
                            rhs=c53[:, 0:HO],
                            start=True,
                            stop=False,
                        )
                        nc.tensor.matmul(
                            out=ps_s[:],
                            lhsT=lmat_sb[:, HO : 2 * HO],
                            rhs=c53[:, HO : 2 * HO],
                            start=False,
                            stop=True,
                        )
                        s54 = spool.tile([HO, WO], dt.bfloat16, tag="s54",
                                         name="s54")
                        nc.vector.tensor_copy(out=s54[:], in_=ps_s[:])
                        s_row = spool.tile([1, HO * WO], dt.bfloat16, tag="s_row",
                                           name="s_row")
                        nc.sync.dma_start(out=s_row[:], in_=s54[:])
                        return s_row

                    # Emission order keeps <=6 PSUM conv groups open and
                    # tracks x chunk arrival.
                    open_groups = []
                    if do_t:
                        emit_t_chunks(range(0, 8))
                    if do_c:
                        pc = emit_taps(0, 0); open_groups.append((0, 0, pc))
                        pc = emit_taps(1, 0); open_groups.append((1, 0, pc))
                    if do_t:
                        emit_t_chunks(range(8, 16))
                    if do_s:
                        emit_t_half(0, 64)
                    if do_c:
                        pc = emit_taps(0, 1); open_groups.append((0, 1, pc))
                        pc = emit_taps(1, 1); open_groups.append((1, 1, pc))
                    if do_t:
                        emit_t_chunks(range(16, 28))
                    s_row = None
                    if do_s:
                        emit_t_half(64, 112)
                        s_row = emit_s_chain()
                    if do_c:
                        pc = emit_taps(0, 2); open_groups.append((0, 2, pc))
                        pc = emit_taps(1, 2); open_groups.append((1, 2, pc))
                    for h2, t2, pc2 in open_groups:
                        emit_bias_and_evict(h2, t2, pc2, s_row)
                    open_groups = []
                    if b + 1 < BPC:
                        emit_x_chunks(b + 1, range(N_XCHUNK))
                    if do_c:
                        for ti in range(3, NTILE):
                            for half in range(2):
                                pc = emit_taps(half, ti)
                                emit_bias_and_evict(half, ti, pc, s_row)

            if iters == 1:
                emit_x_chunks(0, [0])
                bias_sb, lmat_sb, wt_sb = emit_consts()
                emit_x_chunks(0, range(1, N_XCHUNK))
                emit_body(bias_sb, lmat_sb, wt_sb)
            else:
                bias_sb, lmat_sb, wt_sb = emit_consts()
                with tc.For_i(0, iters, 1):
                    emit_x_chunks(0, range(N_XCHUNK))
                    emit_body(bias_sb, lmat_sb, wt_sb)
    nc.finalize()
    return nc


_NC_CACHE = {}


def _get_nc(conv_dt=CONV_DT, iters=1, parts=None):
    key = (str(conv_dt), iters, parts or _PARTS)
    if key not in _NC_CACHE:
        _NC_CACHE[key] = _build(conv_dt, iters, parts)
    return _NC_CACHE[key]


def _host_inputs(input_, weight, bias1, conv_dt=CONV_DT):
    """Build per-core input maps (numpy only)."""
    np_dt = _np_conv_dt(conv_dt)
    input_ = np.asarray(input_, dtype=np.float32).astype(np_dt)
    weight = np.asarray(weight, dtype=np.float32)
    bias1 = np.asarray(bias1, dtype=np.float32)

    wt = np.ascontiguousarray(
        weight.transpose(1, 2, 3, 0).reshape(CIN, 9, COUT)
    ).astype(np_dt)  # [c, (ky kx), o]
    bias_b = bias1.reshape(1, COUT).astype(ml_dtypes.bfloat16)
    lmat = np.zeros((H, 2 * HO), np.float32)
    for y in range(HO):
        for d in range(5):
            lmat[2 * y + d, y] = 0.1
        for d in (0, 2, 4):
            lmat[2 * y + d, HO + y] = -0.1
    ones = np.ones((CIN, 1), np_dt)

    in_maps = []
    for core in range(N_CORES):
        xs = np.ascontiguousarray(input_[core * BPC : (core + 1) * BPC])
        in_maps.append(
            {"x": xs, "wt": wt, "bias": bias_b, "lmat": lmat, "ones": ones}
        )
    return in_maps


def kernel(input_, weight, bias1):
    nc = _get_nc()
    in_maps = _host_inputs(input_, weight, bias1)
    res = run_bass_kernel_spmd(nc, in_maps, core_ids=list(range(N_CORES)))
    out = np.concatenate([r["out"] for r in res.results], axis=0)
    return np.asarray(out, dtype=np.float32)
